# revision 27
# baseline (speedup 1.0000x reference)
"""Trainium2 Bass kernel for DigitConvolutionalModel (conv3x3 + 4-layer MLP).

Strategy:
  - The 3x3 'VALID' conv on 28x28 is a linear map 784->676, so it folds into
    the first linear layer on the host: W1eff[784,1024] = C @ W1.T. The device
    kernel is then a pure 4-layer MLP: relu(x@W1e+b1) -> relu(@W2.T+b2) ->
    relu(@W3.T+b3) -> @W4.T+b4.
  - Pure data parallelism: batch 16384 sharded 8x -> 2048 rows per core.
  - Feature-major layout on device: activations are [features, batch] so each
    layer is out = lhsT.T @ rhs with lhsT = W[in,out] tiles, rhs = h[in, batch].
    Host transposes x shards to [784, 2048]; output comes back [10, 2048].
  - bf16 matmul inputs, fp32 PSUM accumulation, N=512 free dim per matmul
    (one PSUM bank; the ISA caps the matmul free dim at 512). 392 matmuls
    total = the bf16 instruction floor for this network (fp8 DoubleRow was
    measured at 2x bf16 MACs/instr on this hw, which makes the
    accuracy-viable 3-term hi/lo scheme 1.5x SLOWER than bf16 — see
    bench_dr.py/sim_fp8.py).
  - b1/b2/b3 fused into the ScalarE relu (per-partition bias AP); b4 fused
    into the L4 psum->SBUF epilogue (DVE tensor_scalar_add / ScalarE
    Identity+bias), so no ones-row bias matmuls.
  - L4 bt-groups are pipelined into the L3 loop at lag 1 and the last two
    output tiles ship in one fused DMA, so the kernel tail is one epilogue
    op + one DMA issue.
  - Prologue tricks: garbage warmup matmuls (on an uninitialized tile - no
    memset dependency, so they start right at preamble end ~6.8us) release
    the PE HAM clock gate (1.2 -> 2.4 GHz) before real data lands; the
    first k-tile's weights are DMA'd in 4 pieces so the first real matmul
    starts as soon as the DMA ring's ~3us cold-start allows (~10-11us).
"""

import numpy as np
import ml_dtypes
from contextlib import ExitStack

import concourse.mybir as mybir
import concourse.tile as tile
from concourse import bacc
from concourse.bass_utils import run_bass_kernel_spmd

F32 = mybir.dt.float32
BF16 = mybir.dt.bfloat16
AF = mybir.ActivationFunctionType

N_CORES = 8
B = 16384
BC = B // N_CORES          # 2048 rows per core
BT = 512                   # batch tile (free dim per matmul; ISA caps mm free dim at 512)
NBT = BC // BT
K1 = 784                   # 28*28 (conv folded into W1)
D1, D2, D3, D4 = 1024, 512, 256, 10

PS_BUFS = (8 * 2048) // (BT * 4)   # PSUM banks / banks-per-tile
# Layer-1 K is host-padded 784 -> 896 with zero weight rows so all 7 K-tiles
# are uniform [128, *] (non-128 weight loads disable FWL and cost ~+200ns per
# accumulation group). The x tail tile is zero-memset, then rows 0..15 DMA'd.
KORD = [6, 0, 1, 2, 3, 4, 5]       # k6 first: its DMA is tiny, PE starts early

NP_BF16 = ml_dtypes.bfloat16


def _build_nc():
    # Bacc (not plain Bass): its compile pipeline runs
    # generate_event_semaphores, which splits multi-wait instructions (e.g.
    # the kernel-tail drain) into EventSemaphore preludes — TRN2 allows at
    # most one sync wait per instruction.
    nc = bacc.Bacc(None)

    x_d = nc.dram_tensor("x", [K1, BC], BF16, kind="ExternalInput")
    w1_d = nc.dram_tensor("w1", [896, D1], BF16, kind="ExternalInput")
    w2_d = nc.dram_tensor("w2", [D1, D2], BF16, kind="ExternalInput")
    w3_d = nc.dram_tensor("w3", [D2, D3], BF16, kind="ExternalInput")
    w4_d = nc.dram_tensor("w4", [D3, D4], BF16, kind="ExternalInput")
    # bias cols: 0-7 = b1 m-tiles, 8-11 = b2, 12-13 = b3, 14 = b4 (rows 0-9)
    bias_d = nc.dram_tensor("bias", [128, 15], F32, kind="ExternalInput")
    out_d = nc.dram_tensor("out", [D4, BC], F32, kind="ExternalOutput")

    with tile.TileContext(nc) as tc, ExitStack() as ctx:
        sb = ctx.enter_context(tc.tile_pool(name="sb", bufs=1))
        psum = ctx.enter_context(tc.tile_pool(name="psum", bufs=PS_BUFS, space="PSUM"))

        # ---------------- persistent SBUF tiles + DMAs ----------------
        # The sync-engine HWDGE issue stream is serial (~0.7us/DMA), so emit
        # in consumption order: (w1_k, x_k_bt0) pairs in KORD order (k=6 pair
        # is tiny -> first matmul starts as early as possible), then bias,
        # then x for later bts, then later-layer weights.
        xt = [[None] * NBT for _ in range(7)]
        ko = [128 * k for k in range(7)]

        # warmup tile: deliberately UNINITIALIZED (stale SBUF) in the region
        # the warmup matmuls read. They only exist to feed the HAM clock
        # monitor; their psum is never read, and garbage bf16 (even NaN) runs
        # at full rate. No memset dependency means the first warmup issues
        # right at the end of the Tensor engine preamble (~7.2us), firing the
        # HAM ~0.6us earlier. The 1-column memset writes a column the matmuls
        # never read: it allocates the tile without creating a dependency
        # (subtile deps). FULL K=128 partitions: the HAM monitors array
        # utilization — K=1 warmups never fired it.
        warm_sb = sb.tile([128, 128 + BT + 1], BF16, tag="warm", name="warm_sb")
        nc.gpsimd.memset(warm_sb[:, 128 + BT:128 + BT + 1], 1.0)

        def x_tile(k, bt):
            t = sb.tile([128, BT], BF16, tag=f"x_{k}_{bt}", name=f"x_{k}_{bt}")
            if k == 6:
                # rows 16..127 pair with zero weight rows; zero them so
                # uninitialized NaN patterns can't poison 0*x products.
                # Memset on the (idle) DVE queue so it never gates the
                # early Pool/Sync streams. (Tried SWDGE on the GpSimd queue
                # for the start-critical bt0 DMAs: cold-start is no better
                # than the sync HWDGE ring.)
                nc.vector.memset(t[:], 0.0)
                nc.sync.dma_start(out=t[:16, :],
                                  in_=x_d[768:784, bt * BT:(bt + 1) * BT])
            else:
                nc.sync.dma_start(
                    out=t[:], in_=x_d[ko[k]:ko[k] + 128, bt * BT:(bt + 1) * BT])
            return t

        # w1 k-tiles land as pieces (4x256 cols for the first k) so the first
        # matmuls of each k-step start after a partial transfer instead of
        # the full 256KB row block. The start-critical tiles (k6, k0-k2, all
        # x) stay on the SYNC HWDGE ring — the scalar ring's DGE cold-start
        # is ~4-5us, so putting anything early-needed there starves the PE.
        # But the LATE k-tiles (k3-k5, needed ~15-18us) + bias go on the
        # scalar ring: its cold-start completes ~11-13us, in time, and
        # halving the sync ring's front load lands k0-k2/x ~1us earlier.
        k0 = KORD[0]
        w1p = {}
        # piece col-widths per k-tile: tapered for k0 (the first matmul needs
        # only a 32KB transfer), whole 1024-col rows otherwise
        pws = {k: ([128, 128, 256, 512] if k == k0 else [D1]) for k in range(7)}
        for k in KORD:
            off = 0
            rows = 16 if k == 6 else 128   # k6 has 16 real weight rows
            ring = nc.scalar if k in (3, 4, 5) else nc.sync
            for pc, pw in enumerate(pws[k]):
                t = sb.tile([128, pw], BF16, tag=f"w1p_{k}_{pc}", name=f"w1p_{k}_{pc}")
                w1p[(k, pc)] = (t, off)
                if rows < 128:
                    # zero rows come from a POOL memset instead of DMAing
                    # ~200KB of host-side zero padding through the busy
                    # early-DMA window
                    nc.gpsimd.memset(t[:], 0.0)
                ring.dma_start(out=t[:rows, :],
                               in_=w1_d[ko[k]:ko[k] + rows, off:off + pw])
                if pc == 0:
                    xt[k][0] = x_tile(k, 0)
                off += pw

        def w1_slice(k, m):
            c = m * 128
            for pc, pw in enumerate(pws[k]):
                t, off = w1p[(k, pc)]
                if off <= c < off + pw:
                    return t[:, c - off:c - off + 128]
            raise AssertionError

        bias_sb = sb.tile([128, 15], F32, tag="bias", name="bias_sb")
        nc.scalar.dma_start(out=bias_sb[:], in_=bias_d[:])

        for bt in range(1, NBT):
            for k in KORD:
                xt[k][bt] = x_tile(k, bt)

        w2t = []
        for k in range(8):
            t = sb.tile([128, D2], BF16, tag=f"w2_{k}", name=f"w2_{k}")
            nc.sync.dma_start(out=t[:], in_=w2_d[k * 128:(k + 1) * 128, :])
            w2t.append(t)
        w3t = []
        for k in range(4):
            t = sb.tile([128, D3], BF16, tag=f"w3_{k}", name=f"w3_{k}")
            nc.sync.dma_start(out=t[:], in_=w3_d[k * 128:(k + 1) * 128, :])
            w3t.append(t)
        w4t = []
        for k in range(2):
            t = sb.tile([128, D4], BF16, tag=f"w4_{k}", name=f"w4_{k}")
            nc.sync.dma_start(out=t[:], in_=w4_d[k * 128:(k + 1) * 128, :])
            w4t.append(t)

        # activations
        outsb = sb.tile([D4, BC], F32, tag="o", name="o")
        h1 = [[sb.tile([128, BT], BF16, tag=f"h1_{m}_{bt}", name=f"h1_{m}_{bt}")
               for bt in range(NBT)] for m in range(8)]
        h2 = [[sb.tile([128, BT], BF16, tag=f"h2_{m}_{bt}", name=f"h2_{m}_{bt}")
               for bt in range(NBT)] for m in range(4)]
        h3 = [[sb.tile([128, BT], BF16, tag=f"h3_{m}_{bt}", name=f"h3_{m}_{bt}")
               for bt in range(NBT)] for m in range(2)]

        # ---------------- PE warmup ----------------
        # The PE HAM clock gate starts at 1.2 GHz and only releases to
        # 2.4 GHz after ~3.4us of sustained activity. Real matmuls can't
        # start until the first DMAs land (~10us); burn garbage matmuls on an
        # uninitialized SBUF tile from ~7.2us (end of engine preamble) so the
        # HAM fires before/soon after real work begins.
        warm_ps = psum.tile([128, BT], F32, tag="ps", name="warm_ps")
        # The first DMA descriptor has a ~2.8-4us cold-start latency (sync
        # ring: issue ~7.0us, data lands ~10-12us), so real work can't start
        # earlier no matter how early it's issued. 8 warmups at ~430-620ns
        # each bridge preamble-end (~6.8us) to first-data-ready while keeping
        # the PE active, so the HAM releases the full clock (~3.2us of
        # sustained activity) before or right as real matmuls begin.
        for _ in range(8):
            nc.tensor.matmul(warm_ps[:], warm_sb[:, 0:128],
                             warm_sb[:, 128:128 + BT], start=True, stop=True)


        def relu(dst, src, bias_ap, idx):
            # all relus on ScalarE (alternating with DVE tensor_scalar was
            # measured ~0.5us slower: DVE's per-op DRAIN overhead outweighs
            # the ScalarE queue lag it removes)
            nc.scalar.activation(dst, src, AF.Relu, bias=bias_ap)

        def l1_group(p, m, bt):
            for j, k in enumerate(KORD):
                nc.tensor.matmul(
                    p[:], w1_slice(k, m), xt[k][bt][:],
                    start=(j == 0), stop=(j == 6),
                )

        # ---------------- layer 1: [784, BC] -> [1024, BC] ----------------
        # bt0 in k-outer half-passes (PS_BUFS interleaved PSUM groups): the PE
        # consumes each (w1_k, x_k) pair right behind its DMA arrival.
        for half in range(8 // PS_BUFS):
            ms = range(half * PS_BUFS, (half + 1) * PS_BUFS)
            ps0 = {m: psum.tile([128, BT], F32, tag="ps", name=f"ps1_{m}_0")
                   for m in ms}
            for j, k in enumerate(KORD):
                for m in ms:
                    nc.tensor.matmul(
                        ps0[m][:], w1_slice(k, m), xt[k][0][:],
                        start=(j == 0), stop=(j == 6),
                    )
            for m in ms:
                relu(h1[m][0][:], ps0[m][:], bias_sb[:, m:m + 1], m)

        # bt1..: m-outer / k-inner (one PSUM group at a time; relu overlaps)
        for bt in range(1, NBT):
            for m in range(8):
                p = psum.tile([128, BT], F32, tag="ps", name=f"ps1_{m}_{bt}")
                l1_group(p, m, bt)
                relu(h1[m][bt][:], p[:], bias_sb[:, m:m + 1], m)

        # ---------------- layer 2: [1024, BC] -> [512, BC] ----------------
        for bt in range(NBT):
            for m in range(4):
                p = psum.tile([128, BT], F32, tag="ps", name=f"ps2_{m}_{bt}")
                for k in range(8):
                    nc.tensor.matmul(
                        p[:], w2t[k][:, m * 128:(m + 1) * 128], h1[k][bt][:],
                        start=(k == 0), stop=(k == 7),
                    )
                relu(h2[m][bt][:], p[:], bias_sb[:, 8 + m:9 + m], m)

        # ---------------- layers 3+4 pipelined ----------------
        # L4 groups are interleaved into the L3 bt-loop at lag 1, so each
        # L4's h3 relus completed a full L3 group earlier (no PE stall) and
        # only bt3's L4 group trails the last L3 group. Epilogues (psum + b4
        # -> outsb): bt0/bt1/bt2 on the idle DVE, bt3 on ScalarE — the last
        # two run on different engines in parallel, and one fused DMA ships
        # bt2+bt3 together, shortening the kernel tail. (GPSIMD/Pool cannot
        # access PSUM on TRN2; DMA cannot read PSUM either.)
        def l4_group(bt):
            p = psum.tile([D4, BT], F32, tag="ps", name=f"ps4_{bt}")
            nc.tensor.matmul(p[:], w4t[0][:, :], h3[0][bt][:], start=True, stop=False)
            nc.tensor.matmul(p[:], w4t[1][:, :], h3[1][bt][:], start=False, stop=True)
            if bt == NBT - 1:
                nc.scalar.activation(outsb[:, bt * BT:(bt + 1) * BT], p[:],
                                     AF.Identity, bias=bias_sb[:D4, 14:15])
            else:
                nc.vector.tensor_scalar_add(outsb[:, bt * BT:(bt + 1) * BT], p[:],
                                            bias_sb[:D4, 14:15])
            if bt < NBT - 2:
                # early bts stream out during compute
                nc.sync.dma_start(out=out_d[:, bt * BT:(bt + 1) * BT],
                                  in_=outsb[:, bt * BT:(bt + 1) * BT])
            elif bt == NBT - 1:
                # one descriptor for the last two bts (saves a ~0.77us issue
                # from the tail)
                nc.sync.dma_start(out=out_d[:, (NBT - 2) * BT:],
                                  in_=outsb[:, (NBT - 2) * BT:])

        for bt in range(NBT):
            for m in range(2):
                p = psum.tile([128, BT], F32, tag="ps", name=f"ps3_{m}_{bt}")
                for k in range(4):
                    nc.tensor.matmul(
                        p[:], w3t[k][:, m * 128:(m + 1) * 128], h2[k][bt][:],
                        start=(k == 0), stop=(k == 3),
                    )
                relu(h3[m][bt][:], p[:], bias_sb[:, 12 + m:13 + m], m + bt)
            if bt >= 1:
                l4_group(bt - 1)
        l4_group(NBT - 1)

    # run the Bacc pass pipeline (register alloc, wait splitting, ...);
    # run_bass_via_pjrt binds the primitive directly and never finalizes.
    nc.finalize()
    return nc


def _fold_conv(conv_w, W1):
    """W1eff[784,1024] such that x @ W1eff == conv3x3(x, conv_w) @ W1.T."""
    W1img = W1.reshape(D1, 26, 26).transpose(1, 2, 0).astype(np.float32)  # [26,26,1024]
    W1e = np.zeros((28, 28, D1), np.float32)
    for di in range(3):
        for dj in range(3):
            W1e[di:di + 26, dj:dj + 26, :] += np.float32(conv_w[di, dj]) * W1img
    return W1e.reshape(K1, D1)


def _prep_inputs(inputs):
    x = np.asarray(inputs["x"], np.float32)
    conv_w = np.asarray(inputs["conv_w"], np.float32)
    W1 = np.asarray(inputs["W1"], np.float32)
    b1 = np.asarray(inputs["b1"], np.float32)
    W2 = np.asarray(inputs["W2"], np.float32)
    b2 = np.asarray(inputs["b2"], np.float32)
    W3 = np.asarray(inputs["W3"], np.float32)
    b3 = np.asarray(inputs["b3"], np.float32)
    W4 = np.asarray(inputs["W4"], np.float32)
    b4 = np.asarray(inputs["b4"], np.float32)

    w1e = np.zeros((896, D1), np.float32)                          # K padded to 7*128
    w1e[:K1] = _fold_conv(conv_w, W1)
    w1e = w1e.astype(NP_BF16)
    w2 = np.ascontiguousarray(W2.T).astype(NP_BF16)                # [1024, 512]
    w3 = np.ascontiguousarray(W3.T).astype(NP_BF16)                # [512, 256]
    w4 = np.ascontiguousarray(W4.T).astype(NP_BF16)                # [256, 10]
    bias_pack = np.zeros((128, 15), np.float32)
    bias_pack[:, 0:8] = b1.reshape(8, 128).T
    bias_pack[:, 8:12] = b2.reshape(4, 128).T
    bias_pack[:, 12:14] = b3.reshape(2, 128).T
    bias_pack[:D4, 14] = b4

    shared = {"w1": w1e, "w2": w2, "w3": w3, "w4": w4,
              "bias": bias_pack}
    in_maps = []
    for c in range(N_CORES):
        xs = np.ascontiguousarray(x[c * BC:(c + 1) * BC].T).astype(NP_BF16)  # [784, 2048]
        in_maps.append({"x": xs, **shared})
    return in_maps


def _run(inputs, trace=False):
    nc = _build_nc()
    in_maps = _prep_inputs(inputs)
    res = run_bass_kernel_spmd(nc, in_maps, core_ids=list(range(N_CORES)),
                               trace=trace)
    parts = [np.asarray(r["out"], np.float32).T for r in res.results]  # [2048, 10] each
    out = np.concatenate(parts, axis=0)                                # [16384, 10]
    return out, res


def kernel(**inputs):
    out, _ = _run(inputs, trace=False)
    return out



# revision 29
# speedup vs baseline: 1.0172x; 1.0172x over previous
"""Trainium2 Bass kernel for DigitConvolutionalModel (conv3x3 + 4-layer MLP).

Strategy:
  - The 3x3 'VALID' conv on 28x28 is a linear map 784->676, so it folds into
    the first linear layer on the host: W1eff[784,1024] = C @ W1.T. The device
    kernel is then a pure 4-layer MLP: relu(x@W1e+b1) -> relu(@W2.T+b2) ->
    relu(@W3.T+b3) -> @W4.T+b4.
  - Pure data parallelism: batch 16384 sharded 8x -> 2048 rows per core.
  - Feature-major layout on device: activations are [features, batch] so each
    layer is out = lhsT.T @ rhs with lhsT = W[in,out] tiles, rhs = h[in, batch].
    Host transposes x shards to [784, 2048]; output comes back [10, 2048].
  - bf16 matmul inputs, fp32 PSUM accumulation, N=512 free dim per matmul
    (one PSUM bank; the ISA caps the matmul free dim at 512). 392 matmuls
    total = the bf16 instruction floor for this network (fp8 DoubleRow was
    measured at 2x bf16 MACs/instr on this hw, which makes the
    accuracy-viable 3-term hi/lo scheme 1.5x SLOWER than bf16 — see
    bench_dr.py/sim_fp8.py).
  - b1/b2/b3 fused into the ScalarE relu (per-partition bias AP); b4 fused
    into the L4 psum->SBUF epilogue (DVE tensor_scalar_add / ScalarE
    Identity+bias), so no ones-row bias matmuls.
  - L4 bt-groups are pipelined into the L3 loop at lag 1 and the last two
    output tiles ship in one fused DMA, so the kernel tail is one epilogue
    op + one DMA issue.
  - Prologue tricks: garbage warmup matmuls (on an uninitialized tile - no
    memset dependency, so they start right at preamble end ~6.8us) release
    the PE HAM clock gate (1.2 -> 2.4 GHz) before real data lands; the
    first k-tile's weights are DMA'd in 4 pieces so the first real matmul
    starts as soon as the DMA ring's ~3us cold-start allows (~10-11us).
"""

import numpy as np
import ml_dtypes
from contextlib import ExitStack

import concourse.mybir as mybir
import concourse.tile as tile
from concourse import bacc
from concourse.bass_utils import run_bass_kernel_spmd

F32 = mybir.dt.float32
BF16 = mybir.dt.bfloat16
AF = mybir.ActivationFunctionType

N_CORES = 8
B = 16384
BC = B // N_CORES          # 2048 rows per core
BT = 512                   # batch tile (free dim per matmul; ISA caps mm free dim at 512)
NBT = BC // BT
K1 = 784                   # 28*28 (conv folded into W1)
D1, D2, D3, D4 = 1024, 512, 256, 10

PS_BUFS = (8 * 2048) // (BT * 4)   # PSUM banks / banks-per-tile
# Layer-1 K is host-padded 784 -> 896 with zero weight rows so all 7 K-tiles
# are uniform [128, *] (non-128 weight loads disable FWL and cost ~+200ns per
# accumulation group). The x tail tile is zero-memset, then rows 0..15 DMA'd.
KORD = [6, 0, 1, 2, 3, 4, 5]       # k6 first: its DMA is tiny, PE starts early

NP_BF16 = ml_dtypes.bfloat16


def _build_nc():
    # Bacc (not plain Bass): its compile pipeline runs
    # generate_event_semaphores, which splits multi-wait instructions (e.g.
    # the kernel-tail drain) into EventSemaphore preludes — TRN2 allows at
    # most one sync wait per instruction.
    nc = bacc.Bacc(None)

    x_d = nc.dram_tensor("x", [K1, BC], BF16, kind="ExternalInput")
    w1_d = nc.dram_tensor("w1", [896, D1], BF16, kind="ExternalInput")
    w2_d = nc.dram_tensor("w2", [D1, D2], BF16, kind="ExternalInput")
    w3_d = nc.dram_tensor("w3", [D2, D3], BF16, kind="ExternalInput")
    w4_d = nc.dram_tensor("w4", [D3, D4], BF16, kind="ExternalInput")
    # bias cols: 0-7 = b1 m-tiles, 8-11 = b2, 12-13 = b3, 14 = b4 (rows 0-9)
    bias_d = nc.dram_tensor("bias", [128, 15], F32, kind="ExternalInput")
    out_d = nc.dram_tensor("out", [D4, BC], F32, kind="ExternalOutput")

    with tile.TileContext(nc) as tc, ExitStack() as ctx:
        sb = ctx.enter_context(tc.tile_pool(name="sb", bufs=1))
        psum = ctx.enter_context(tc.tile_pool(name="psum", bufs=PS_BUFS, space="PSUM"))

        # ---------------- persistent SBUF tiles + DMAs ----------------
        # The sync-engine HWDGE issue stream is serial (~0.7us/DMA), so emit
        # in consumption order: (w1_k, x_k_bt0) pairs in KORD order (k=6 pair
        # is tiny -> first matmul starts as early as possible), then bias,
        # then x for later bts, then later-layer weights.
        xt = [[None] * NBT for _ in range(7)]
        ko = [128 * k for k in range(7)]

        # warmup tile: deliberately UNINITIALIZED (stale SBUF) in the region
        # the warmup matmuls read. They only exist to feed the HAM clock
        # monitor; their psum is never read, and garbage bf16 (even NaN) runs
        # at full rate. No memset dependency means the first warmup issues
        # right at the end of the Tensor engine preamble (~7.2us), firing the
        # HAM ~0.6us earlier. The 1-column memset writes a column the matmuls
        # never read: it allocates the tile without creating a dependency
        # (subtile deps). FULL K=128 partitions: the HAM monitors array
        # utilization — K=1 warmups never fired it.
        warm_sb = sb.tile([128, 128 + BT + 1], BF16, tag="warm", name="warm_sb")
        nc.gpsimd.memset(warm_sb[:, 128 + BT:128 + BT + 1], 1.0)

        def x_tile(k, bt):
            t = sb.tile([128, BT], BF16, tag=f"x_{k}_{bt}", name=f"x_{k}_{bt}")
            if k == 6:
                # rows 16..127 pair with zero weight rows; zero them so
                # uninitialized NaN patterns can't poison 0*x products.
                # Memset on the (idle) DVE queue so it never gates the
                # early Pool/Sync streams. (Tried SWDGE on the GpSimd queue
                # for the start-critical bt0 DMAs: cold-start is no better
                # than the sync HWDGE ring.)
                nc.vector.memset(t[:], 0.0)
                nc.sync.dma_start(out=t[:16, :],
                                  in_=x_d[768:784, bt * BT:(bt + 1) * BT])
            else:
                nc.sync.dma_start(
                    out=t[:], in_=x_d[ko[k]:ko[k] + 128, bt * BT:(bt + 1) * BT])
            return t

        # w1 k-tiles land as pieces (4x256 cols for the first k) so the first
        # matmuls of each k-step start after a partial transfer instead of
        # the full 256KB row block. Everything stays on the SYNC HWDGE ring:
        # the scalar ring was measured worse BOTH for start-critical tiles
        # (its DGE cold-start is ~4-5us) AND for the late k3-k5 tiles
        # (1-1.7us PE stalls at their consumption — late delivery/HBM
        # contention at the front). Single ring wins; L1's start is gated by
        # first-data cold-start, not issue throughput.
        k0 = KORD[0]
        w1p = {}
        # piece col-widths per k-tile: tapered for k0 (the first matmul needs
        # only a 32KB transfer), whole 1024-col rows otherwise
        pws = {k: ([128, 128, 256, 512] if k == k0 else [D1]) for k in range(7)}
        for k in KORD:
            off = 0
            rows = 16 if k == 6 else 128   # k6 has 16 real weight rows
            for pc, pw in enumerate(pws[k]):
                t = sb.tile([128, pw], BF16, tag=f"w1p_{k}_{pc}", name=f"w1p_{k}_{pc}")
                w1p[(k, pc)] = (t, off)
                if rows < 128:
                    # zero rows come from a POOL memset instead of DMAing
                    # ~200KB of host-side zero padding through the busy
                    # early-DMA window
                    nc.gpsimd.memset(t[:], 0.0)
                nc.sync.dma_start(out=t[:rows, :],
                                  in_=w1_d[ko[k]:ko[k] + rows, off:off + pw])
                if pc == 0:
                    xt[k][0] = x_tile(k, 0)
                off += pw

        def w1_slice(k, m):
            c = m * 128
            for pc, pw in enumerate(pws[k]):
                t, off = w1p[(k, pc)]
                if off <= c < off + pw:
                    return t[:, c - off:c - off + 128]
            raise AssertionError

        bias_sb = sb.tile([128, 15], F32, tag="bias", name="bias_sb")
        nc.sync.dma_start(out=bias_sb[:], in_=bias_d[:])

        for bt in range(1, NBT):
            for k in KORD:
                xt[k][bt] = x_tile(k, bt)

        w2t = []
        for k in range(8):
            t = sb.tile([128, D2], BF16, tag=f"w2_{k}", name=f"w2_{k}")
            nc.sync.dma_start(out=t[:], in_=w2_d[k * 128:(k + 1) * 128, :])
            w2t.append(t)
        w3t = []
        for k in range(4):
            t = sb.tile([128, D3], BF16, tag=f"w3_{k}", name=f"w3_{k}")
            nc.sync.dma_start(out=t[:], in_=w3_d[k * 128:(k + 1) * 128, :])
            w3t.append(t)
        w4t = []
        for k in range(2):
            t = sb.tile([128, D4], BF16, tag=f"w4_{k}", name=f"w4_{k}")
            nc.sync.dma_start(out=t[:], in_=w4_d[k * 128:(k + 1) * 128, :])
            w4t.append(t)

        # activations
        outsb = sb.tile([D4, BC], F32, tag="o", name="o")
        h1 = [[sb.tile([128, BT], BF16, tag=f"h1_{m}_{bt}", name=f"h1_{m}_{bt}")
               for bt in range(NBT)] for m in range(8)]
        h2 = [[sb.tile([128, BT], BF16, tag=f"h2_{m}_{bt}", name=f"h2_{m}_{bt}")
               for bt in range(NBT)] for m in range(4)]
        h3 = [[sb.tile([128, BT], BF16, tag=f"h3_{m}_{bt}", name=f"h3_{m}_{bt}")
               for bt in range(NBT)] for m in range(2)]

        # ---------------- PE warmup ----------------
        # The PE HAM clock gate starts at 1.2 GHz and only releases to
        # 2.4 GHz after ~3.4us of sustained activity. Real matmuls can't
        # start until the first DMAs land (~10us); burn garbage matmuls on an
        # uninitialized SBUF tile from ~7.2us (end of engine preamble) so the
        # HAM fires before/soon after real work begins.
        warm_ps = psum.tile([128, BT], F32, tag="ps", name="warm_ps")
        # The first DMA descriptor has a ~2.8-4us cold-start latency (sync
        # ring: issue ~7.0us, data lands ~10-12us), so real work can't start
        # earlier no matter how early it's issued. 8 warmups at ~430-620ns
        # each bridge preamble-end (~6.8us) to first-data-ready while keeping
        # the PE active, so the HAM releases the full clock (~3.2us of
        # sustained activity) before or right as real matmuls begin.
        for _ in range(8):
            nc.tensor.matmul(warm_ps[:], warm_sb[:, 0:128],
                             warm_sb[:, 128:128 + BT], start=True, stop=True)


        def relu(dst, src, bias_ap, idx):
            # all relus on ScalarE (alternating with DVE tensor_scalar was
            # measured ~0.5us slower: DVE's per-op DRAIN overhead outweighs
            # the ScalarE queue lag it removes)
            nc.scalar.activation(dst, src, AF.Relu, bias=bias_ap)

        def l1_group(p, m, bt):
            for j, k in enumerate(KORD):
                nc.tensor.matmul(
                    p[:], w1_slice(k, m), xt[k][bt][:],
                    start=(j == 0), stop=(j == 6),
                )

        # ---------------- layer 1: [784, BC] -> [1024, BC] ----------------
        # bt0 in k-outer half-passes (PS_BUFS interleaved PSUM groups): the PE
        # consumes each (w1_k, x_k) pair right behind its DMA arrival.
        for half in range(8 // PS_BUFS):
            ms = range(half * PS_BUFS, (half + 1) * PS_BUFS)
            ps0 = {m: psum.tile([128, BT], F32, tag="ps", name=f"ps1_{m}_0")
                   for m in ms}
            for j, k in enumerate(KORD):
                for m in ms:
                    nc.tensor.matmul(
                        ps0[m][:], w1_slice(k, m), xt[k][0][:],
                        start=(j == 0), stop=(j == 6),
                    )
            for m in ms:
                relu(h1[m][0][:], ps0[m][:], bias_sb[:, m:m + 1], m)

        # bt1..: m-outer / k-inner (one PSUM group at a time; relu overlaps)
        for bt in range(1, NBT):
            for m in range(8):
                p = psum.tile([128, BT], F32, tag="ps", name=f"ps1_{m}_{bt}")
                l1_group(p, m, bt)
                relu(h1[m][bt][:], p[:], bias_sb[:, m:m + 1], m)

        # ---------------- layer 2: [1024, BC] -> [512, BC] ----------------
        for bt in range(NBT):
            for m in range(4):
                p = psum.tile([128, BT], F32, tag="ps", name=f"ps2_{m}_{bt}")
                for k in range(8):
                    nc.tensor.matmul(
                        p[:], w2t[k][:, m * 128:(m + 1) * 128], h1[k][bt][:],
                        start=(k == 0), stop=(k == 7),
                    )
                relu(h2[m][bt][:], p[:], bias_sb[:, 8 + m:9 + m], m)

        # ---------------- layers 3+4 pipelined ----------------
        # L4 groups are interleaved into the L3 bt-loop at lag 1, so each
        # L4's h3 relus completed a full L3 group earlier (no PE stall) and
        # only bt3's L4 group trails the last L3 group. Epilogues (psum + b4
        # -> outsb): bt0/bt1/bt2 on the idle DVE, bt3 on ScalarE — the last
        # two run on different engines in parallel, and one fused DMA ships
        # bt2+bt3 together, shortening the kernel tail. (GPSIMD/Pool cannot
        # access PSUM on TRN2; DMA cannot read PSUM either.)
        def l4_group(bt):
            p = psum.tile([D4, BT], F32, tag="ps", name=f"ps4_{bt}")
            nc.tensor.matmul(p[:], w4t[0][:, :], h3[0][bt][:], start=True, stop=False)
            nc.tensor.matmul(p[:], w4t[1][:, :], h3[1][bt][:], start=False, stop=True)
            if bt == NBT - 1:
                nc.scalar.activation(outsb[:, bt * BT:(bt + 1) * BT], p[:],
                                     AF.Identity, bias=bias_sb[:D4, 14:15])
            else:
                nc.vector.tensor_scalar_add(outsb[:, bt * BT:(bt + 1) * BT], p[:],
                                            bias_sb[:D4, 14:15])
            if bt < NBT - 2:
                # early bts stream out during compute
                nc.sync.dma_start(out=out_d[:, bt * BT:(bt + 1) * BT],
                                  in_=outsb[:, bt * BT:(bt + 1) * BT])
            elif bt == NBT - 1:
                # one descriptor for the last two bts (saves a ~0.77us issue
                # from the tail)
                nc.sync.dma_start(out=out_d[:, (NBT - 2) * BT:],
                                  in_=outsb[:, (NBT - 2) * BT:])

        for bt in range(NBT):
            for m in range(2):
                p = psum.tile([128, BT], F32, tag="ps", name=f"ps3_{m}_{bt}")
                for k in range(4):
                    nc.tensor.matmul(
                        p[:], w3t[k][:, m * 128:(m + 1) * 128], h2[k][bt][:],
                        start=(k == 0), stop=(k == 3),
                    )
                relu(h3[m][bt][:], p[:], bias_sb[:, 12 + m:13 + m], m + bt)
            if bt >= 1:
                l4_group(bt - 1)
        l4_group(NBT - 1)

    # run the Bacc pass pipeline (register alloc, wait splitting, ...);
    # run_bass_via_pjrt binds the primitive directly and never finalizes.
    nc.finalize()
    return nc


def _fold_conv(conv_w, W1):
    """W1eff[784,1024] such that x @ W1eff == conv3x3(x, conv_w) @ W1.T."""
    W1img = W1.reshape(D1, 26, 26).transpose(1, 2, 0).astype(np.float32)  # [26,26,1024]
    W1e = np.zeros((28, 28, D1), np.float32)
    for di in range(3):
        for dj in range(3):
            W1e[di:di + 26, dj:dj + 26, :] += np.float32(conv_w[di, dj]) * W1img
    return W1e.reshape(K1, D1)


def _prep_inputs(inputs):
    x = np.asarray(inputs["x"], np.float32)
    conv_w = np.asarray(inputs["conv_w"], np.float32)
    W1 = np.asarray(inputs["W1"], np.float32)
    b1 = np.asarray(inputs["b1"], np.float32)
    W2 = np.asarray(inputs["W2"], np.float32)
    b2 = np.asarray(inputs["b2"], np.float32)
    W3 = np.asarray(inputs["W3"], np.float32)
    b3 = np.asarray(inputs["b3"], np.float32)
    W4 = np.asarray(inputs["W4"], np.float32)
    b4 = np.asarray(inputs["b4"], np.float32)

    w1e = np.zeros((896, D1), np.float32)                          # K padded to 7*128
    w1e[:K1] = _fold_conv(conv_w, W1)
    w1e = w1e.astype(NP_BF16)
    w2 = np.ascontiguousarray(W2.T).astype(NP_BF16)                # [1024, 512]
    w3 = np.ascontiguousarray(W3.T).astype(NP_BF16)                # [512, 256]
    w4 = np.ascontiguousarray(W4.T).astype(NP_BF16)                # [256, 10]
    bias_pack = np.zeros((128, 15), np.float32)
    bias_pack[:, 0:8] = b1.reshape(8, 128).T
    bias_pack[:, 8:12] = b2.reshape(4, 128).T
    bias_pack[:, 12:14] = b3.reshape(2, 128).T
    bias_pack[:D4, 14] = b4

    shared = {"w1": w1e, "w2": w2, "w3": w3, "w4": w4,
              "bias": bias_pack}
    in_maps = []
    for c in range(N_CORES):
        xs = np.ascontiguousarray(x[c * BC:(c + 1) * BC].T).astype(NP_BF16)  # [784, 2048]
        in_maps.append({"x": xs, **shared})
    return in_maps


def _run(inputs, trace=False):
    nc = _build_nc()
    in_maps = _prep_inputs(inputs)
    res = run_bass_kernel_spmd(nc, in_maps, core_ids=list(range(N_CORES)),
                               trace=trace)
    parts = [np.asarray(r["out"], np.float32).T for r in res.results]  # [2048, 10] each
    out = np.concatenate(parts, axis=0)                                # [16384, 10]
    return out, res


def kernel(**inputs):
    out, _ = _run(inputs, trace=False)
    return out



# revision 31
# speedup vs baseline: 1.0322x; 1.0147x over previous
"""Trainium2 Bass kernel for DigitConvolutionalModel (conv3x3 + 4-layer MLP).

Strategy:
  - The 3x3 'VALID' conv on 28x28 is a linear map 784->676, so it folds into
    the first linear layer on the host: W1eff[784,1024] = C @ W1.T. The device
    kernel is then a pure 4-layer MLP: relu(x@W1e+b1) -> relu(@W2.T+b2) ->
    relu(@W3.T+b3) -> @W4.T+b4.
  - Pure data parallelism: batch 16384 sharded 8x -> 2048 rows per core.
  - Feature-major layout on device: activations are [features, batch] so each
    layer is out = lhsT.T @ rhs with lhsT = W[in,out] tiles, rhs = h[in, batch].
    Host transposes x shards to [784, 2048]; output comes back [10, 2048].
  - bf16 matmul inputs, fp32 PSUM accumulation, N=512 free dim per matmul
    (one PSUM bank; the ISA caps the matmul free dim at 512). 392 matmuls
    total = the bf16 instruction floor for this network (fp8 DoubleRow was
    measured at 2x bf16 MACs/instr on this hw, which makes the
    accuracy-viable 3-term hi/lo scheme 1.5x SLOWER than bf16 — see
    bench_dr.py/sim_fp8.py).
  - b1/b2/b3 fused into the ScalarE relu (per-partition bias AP); b4 fused
    into the L4 psum->SBUF epilogue (DVE tensor_scalar_add / ScalarE
    Identity+bias), so no ones-row bias matmuls.
  - L4 bt-groups are pipelined into the L3 loop at lag 1 and the last two
    output tiles ship in one fused DMA, so the kernel tail is one epilogue
    op + one DMA issue.
  - Prologue tricks: garbage warmup matmuls (on an uninitialized tile - no
    memset dependency, so they start right at preamble end ~6.8us) release
    the PE HAM clock gate (1.2 -> 2.4 GHz) before real data lands; the
    first k-tile's weights are DMA'd in 4 pieces so the first real matmul
    starts as soon as the DMA ring's ~3us cold-start allows (~10-11us).
"""

import numpy as np
import ml_dtypes
from contextlib import ExitStack

import concourse.mybir as mybir
import concourse.tile as tile
from concourse import bacc
from concourse.bass_utils import run_bass_kernel_spmd

F32 = mybir.dt.float32
BF16 = mybir.dt.bfloat16
AF = mybir.ActivationFunctionType

N_CORES = 8
B = 16384
BC = B // N_CORES          # 2048 rows per core
BT = 512                   # batch tile (free dim per matmul; ISA caps mm free dim at 512)
NBT = BC // BT
K1 = 784                   # 28*28 (conv folded into W1)
D1, D2, D3, D4 = 1024, 512, 256, 10

PS_BUFS = (8 * 2048) // (BT * 4)   # PSUM banks / banks-per-tile
# Layer-1 K is host-padded 784 -> 896 with zero weight rows so all 7 K-tiles
# are uniform [128, *] (non-128 weight loads disable FWL and cost ~+200ns per
# accumulation group). The x tail tile is zero-memset, then rows 0..15 DMA'd.
KORD = [6, 0, 1, 2, 3, 4, 5]       # k6 first: its DMA is tiny, PE starts early

NP_BF16 = ml_dtypes.bfloat16


def _build_nc():
    # Bacc (not plain Bass): its compile pipeline runs
    # generate_event_semaphores, which splits multi-wait instructions (e.g.
    # the kernel-tail drain) into EventSemaphore preludes — TRN2 allows at
    # most one sync wait per instruction.
    nc = bacc.Bacc(None)

    x_d = nc.dram_tensor("x", [K1, BC], BF16, kind="ExternalInput")
    w1_d = nc.dram_tensor("w1", [896, D1], BF16, kind="ExternalInput")
    w2_d = nc.dram_tensor("w2", [D1, D2], BF16, kind="ExternalInput")
    w3_d = nc.dram_tensor("w3", [D2, D3], BF16, kind="ExternalInput")
    w4_d = nc.dram_tensor("w4", [D3, D4], BF16, kind="ExternalInput")
    # bias cols: 0-7 = b1 m-tiles, 8-11 = b2, 12-13 = b3, 14 = b4 (rows 0-9)
    bias_d = nc.dram_tensor("bias", [128, 15], F32, kind="ExternalInput")
    out_d = nc.dram_tensor("out", [D4, BC], F32, kind="ExternalOutput")

    with tile.TileContext(nc) as tc, ExitStack() as ctx:
        sb = ctx.enter_context(tc.tile_pool(name="sb", bufs=1))
        psum = ctx.enter_context(tc.tile_pool(name="psum", bufs=PS_BUFS, space="PSUM"))

        # ---------------- persistent SBUF tiles + DMAs ----------------
        # The sync-engine HWDGE issue stream is serial (~0.7us/DMA), so emit
        # in consumption order: (w1_k, x_k_bt0) pairs in KORD order (k=6 pair
        # is tiny -> first matmul starts as early as possible), then bias,
        # then x for later bts, then later-layer weights.
        xt = [[None] * NBT for _ in range(7)]
        ko = [128 * k for k in range(7)]

        # warmup tile: deliberately UNINITIALIZED (stale SBUF) in the region
        # the warmup matmuls read. They only exist to feed the HAM clock
        # monitor; their psum is never read, and garbage bf16 (even NaN) runs
        # at full rate. No memset dependency means the first warmup issues
        # right at the end of the Tensor engine preamble (~7.2us), firing the
        # HAM ~0.6us earlier. The 1-column memset writes a column the matmuls
        # never read: it allocates the tile without creating a dependency
        # (subtile deps). FULL K=128 partitions: the HAM monitors array
        # utilization — K=1 warmups never fired it.
        warm_sb = sb.tile([128, 128 + BT + 1], BF16, tag="warm", name="warm_sb")
        nc.gpsimd.memset(warm_sb[:, 128 + BT:128 + BT + 1], 1.0)

        def x_tile(k, bt):
            t = sb.tile([128, BT], BF16, tag=f"x_{k}_{bt}", name=f"x_{k}_{bt}")
            if k == 6:
                # rows 16..127 pair with zero weight rows; zero them so
                # uninitialized NaN patterns can't poison 0*x products.
                # Memset on the (idle) DVE queue so it never gates the
                # early Pool/Sync streams. (Tried SWDGE on the GpSimd queue
                # for the start-critical bt0 DMAs: cold-start is no better
                # than the sync HWDGE ring.)
                nc.vector.memset(t[:], 0.0)
                nc.sync.dma_start(out=t[:16, :],
                                  in_=x_d[768:784, bt * BT:(bt + 1) * BT])
            else:
                nc.sync.dma_start(
                    out=t[:], in_=x_d[ko[k]:ko[k] + 128, bt * BT:(bt + 1) * BT])
            return t

        # w1 k-tiles land as pieces (4x256 cols for the first k) so the first
        # matmuls of each k-step start after a partial transfer instead of
        # the full 256KB row block. Everything stays on the SYNC HWDGE ring:
        # the scalar ring was measured worse BOTH for start-critical tiles
        # (its DGE cold-start is ~4-5us) AND for the late k3-k5 tiles
        # (1-1.7us PE stalls at their consumption — late delivery/HBM
        # contention at the front). Single ring wins; L1's start is gated by
        # first-data cold-start, not issue throughput.
        k0 = KORD[0]
        w1p = {}
        # piece col-widths per k-tile: tapered for k0 (the first matmul needs
        # only a 32KB transfer), whole 1024-col rows otherwise
        pws = {k: ([128, 128, 256, 512] if k == k0 else [D1]) for k in range(7)}
        for k in KORD:
            off = 0
            rows = 16 if k == 6 else 128   # k6 has 16 real weight rows
            for pc, pw in enumerate(pws[k]):
                t = sb.tile([128, pw], BF16, tag=f"w1p_{k}_{pc}", name=f"w1p_{k}_{pc}")
                w1p[(k, pc)] = (t, off)
                if rows < 128:
                    # zero rows come from a POOL memset instead of DMAing
                    # ~200KB of host-side zero padding through the busy
                    # early-DMA window
                    nc.gpsimd.memset(t[:], 0.0)
                nc.sync.dma_start(out=t[:rows, :],
                                  in_=w1_d[ko[k]:ko[k] + rows, off:off + pw])
                if pc == 0:
                    xt[k][0] = x_tile(k, 0)
                off += pw

        def w1_slice(k, m):
            c = m * 128
            for pc, pw in enumerate(pws[k]):
                t, off = w1p[(k, pc)]
                if off <= c < off + pw:
                    return t[:, c - off:c - off + 128]
            raise AssertionError

        bias_sb = sb.tile([128, 15], F32, tag="bias", name="bias_sb")
        nc.sync.dma_start(out=bias_sb[:], in_=bias_d[:])

        for bt in range(1, NBT):
            for k in KORD:
                xt[k][bt] = x_tile(k, bt)

        w2t = []
        for k in range(8):
            t = sb.tile([128, D2], BF16, tag=f"w2_{k}", name=f"w2_{k}")
            nc.sync.dma_start(out=t[:], in_=w2_d[k * 128:(k + 1) * 128, :])
            w2t.append(t)
        w3t = []
        for k in range(4):
            t = sb.tile([128, D3], BF16, tag=f"w3_{k}", name=f"w3_{k}")
            nc.sync.dma_start(out=t[:], in_=w3_d[k * 128:(k + 1) * 128, :])
            w3t.append(t)
        w4t = []
        for k in range(2):
            t = sb.tile([128, D4], BF16, tag=f"w4_{k}", name=f"w4_{k}")
            nc.sync.dma_start(out=t[:], in_=w4_d[k * 128:(k + 1) * 128, :])
            w4t.append(t)

        # activations
        outsb = sb.tile([D4, BC], F32, tag="o", name="o")
        h1 = [[sb.tile([128, BT], BF16, tag=f"h1_{m}_{bt}", name=f"h1_{m}_{bt}")
               for bt in range(NBT)] for m in range(8)]
        h2 = [[sb.tile([128, BT], BF16, tag=f"h2_{m}_{bt}", name=f"h2_{m}_{bt}")
               for bt in range(NBT)] for m in range(4)]
        h3 = [[sb.tile([128, BT], BF16, tag=f"h3_{m}_{bt}", name=f"h3_{m}_{bt}")
               for bt in range(NBT)] for m in range(2)]

        # ---------------- PE warmup ----------------
        # The PE HAM clock gate starts at 1.2 GHz and only releases to
        # 2.4 GHz after ~3.4us of sustained activity. Real matmuls can't
        # start until the first DMAs land (~10us); burn garbage matmuls on an
        # uninitialized SBUF tile from ~7.2us (end of engine preamble) so the
        # HAM fires before/soon after real work begins.
        warm_ps = psum.tile([128, BT], F32, tag="ps", name="warm_ps")
        # The first DMA descriptor has a ~2.8-4us cold-start latency (sync
        # ring: issue ~7.0us, data lands ~10-12us), so real work can't start
        # earlier no matter how early it's issued. 8 warmups at ~430-620ns
        # each bridge preamble-end (~6.8us) to first-data-ready while keeping
        # the PE active, so the HAM releases the full clock (~3.2us of
        # sustained activity) before or right as real matmuls begin.
        for _ in range(8):
            nc.tensor.matmul(warm_ps[:], warm_sb[:, 0:128],
                             warm_sb[:, 128:128 + BT], start=True, stop=True)


        def relu(dst, src, bias_ap, idx):
            # all relus on ScalarE (alternating with DVE tensor_scalar was
            # measured ~0.5us slower: DVE's per-op DRAIN overhead outweighs
            # the ScalarE queue lag it removes)
            nc.scalar.activation(dst, src, AF.Relu, bias=bias_ap)

        def l1_group(p, m, bt):
            for j, k in enumerate(KORD):
                nc.tensor.matmul(
                    p[:], w1_slice(k, m), xt[k][bt][:],
                    start=(j == 0), stop=(j == 6),
                )

        # ---------------- layer 1: [784, BC] -> [1024, BC] ----------------
        # bt0 in k-outer half-passes (PS_BUFS interleaved PSUM groups): the PE
        # consumes each (w1_k, x_k) pair right behind its DMA arrival.
        for half in range(8 // PS_BUFS):
            ms = range(half * PS_BUFS, (half + 1) * PS_BUFS)
            ps0 = {m: psum.tile([128, BT], F32, tag="ps", name=f"ps1_{m}_0")
                   for m in ms}
            for j, k in enumerate(KORD):
                for m in ms:
                    nc.tensor.matmul(
                        ps0[m][:], w1_slice(k, m), xt[k][0][:],
                        start=(j == 0), stop=(j == 6),
                    )
            for m in ms:
                relu(h1[m][0][:], ps0[m][:], bias_sb[:, m:m + 1], m)

        # bt1..: m-outer / k-inner (one PSUM group at a time; relu overlaps)
        for bt in range(1, NBT):
            for m in range(8):
                p = psum.tile([128, BT], F32, tag="ps", name=f"ps1_{m}_{bt}")
                l1_group(p, m, bt)
                relu(h1[m][bt][:], p[:], bias_sb[:, m:m + 1], m)

        # ---------------- layer 2: [1024, BC] -> [512, BC] ----------------
        for bt in range(NBT):
            for m in range(4):
                p = psum.tile([128, BT], F32, tag="ps", name=f"ps2_{m}_{bt}")
                for k in range(8):
                    nc.tensor.matmul(
                        p[:], w2t[k][:, m * 128:(m + 1) * 128], h1[k][bt][:],
                        start=(k == 0), stop=(k == 7),
                    )
                relu(h2[m][bt][:], p[:], bias_sb[:, 8 + m:9 + m], m)

        # ---------------- layers 3+4 pipelined ----------------
        # L4 groups are interleaved into the L3 bt-loop at lag 2: each mid
        # L4's h3 relus completed two full L3 groups earlier, and the two
        # trailing L4 groups (bt2, bt3) back-to-back give relu(b3m1) 0.86us
        # of matmul cover (vs its ~0.9us lag), eliminating both the mid
        # 0.4-0.5us stalls and most of the final one. Epilogues (psum + b4
        # -> outsb): bt0/bt1/bt2 on the idle DVE, bt3 on ScalarE — the last
        # two run on different engines in parallel, and one fused DMA ships
        # bt2+bt3 together, shortening the kernel tail. (GPSIMD/Pool cannot
        # access PSUM on TRN2; DMA cannot read PSUM either.)
        def l4_group(bt):
            p = psum.tile([D4, BT], F32, tag="ps", name=f"ps4_{bt}")
            nc.tensor.matmul(p[:], w4t[0][:, :], h3[0][bt][:], start=True, stop=False)
            nc.tensor.matmul(p[:], w4t[1][:, :], h3[1][bt][:], start=False, stop=True)
            if bt == NBT - 1:
                nc.scalar.activation(outsb[:, bt * BT:(bt + 1) * BT], p[:],
                                     AF.Identity, bias=bias_sb[:D4, 14:15])
            else:
                nc.vector.tensor_scalar_add(outsb[:, bt * BT:(bt + 1) * BT], p[:],
                                            bias_sb[:D4, 14:15])
            if bt < NBT - 2:
                # early bts stream out during compute
                nc.sync.dma_start(out=out_d[:, bt * BT:(bt + 1) * BT],
                                  in_=outsb[:, bt * BT:(bt + 1) * BT])
            elif bt == NBT - 1:
                # one descriptor for the last two bts (saves a ~0.77us issue
                # from the tail)
                nc.sync.dma_start(out=out_d[:, (NBT - 2) * BT:],
                                  in_=outsb[:, (NBT - 2) * BT:])

        for bt in range(NBT):
            for m in range(2):
                p = psum.tile([128, BT], F32, tag="ps", name=f"ps3_{m}_{bt}")
                for k in range(4):
                    nc.tensor.matmul(
                        p[:], w3t[k][:, m * 128:(m + 1) * 128], h2[k][bt][:],
                        start=(k == 0), stop=(k == 3),
                    )
                relu(h3[m][bt][:], p[:], bias_sb[:, 12 + m:13 + m], m + bt)
            if bt >= 2:
                l4_group(bt - 2)
        l4_group(NBT - 2)
        l4_group(NBT - 1)

    # run the Bacc pass pipeline (register alloc, wait splitting, ...);
    # run_bass_via_pjrt binds the primitive directly and never finalizes.
    nc.finalize()
    return nc


def _fold_conv(conv_w, W1):
    """W1eff[784,1024] such that x @ W1eff == conv3x3(x, conv_w) @ W1.T."""
    W1img = W1.reshape(D1, 26, 26).transpose(1, 2, 0).astype(np.float32)  # [26,26,1024]
    W1e = np.zeros((28, 28, D1), np.float32)
    for di in range(3):
        for dj in range(3):
            W1e[di:di + 26, dj:dj + 26, :] += np.float32(conv_w[di, dj]) * W1img
    return W1e.reshape(K1, D1)


def _prep_inputs(inputs):
    x = np.asarray(inputs["x"], np.float32)
    conv_w = np.asarray(inputs["conv_w"], np.float32)
    W1 = np.asarray(inputs["W1"], np.float32)
    b1 = np.asarray(inputs["b1"], np.float32)
    W2 = np.asarray(inputs["W2"], np.float32)
    b2 = np.asarray(inputs["b2"], np.float32)
    W3 = np.asarray(inputs["W3"], np.float32)
    b3 = np.asarray(inputs["b3"], np.float32)
    W4 = np.asarray(inputs["W4"], np.float32)
    b4 = np.asarray(inputs["b4"], np.float32)

    w1e = np.zeros((896, D1), np.float32)                          # K padded to 7*128
    w1e[:K1] = _fold_conv(conv_w, W1)
    w1e = w1e.astype(NP_BF16)
    w2 = np.ascontiguousarray(W2.T).astype(NP_BF16)                # [1024, 512]
    w3 = np.ascontiguousarray(W3.T).astype(NP_BF16)                # [512, 256]
    w4 = np.ascontiguousarray(W4.T).astype(NP_BF16)                # [256, 10]
    bias_pack = np.zeros((128, 15), np.float32)
    bias_pack[:, 0:8] = b1.reshape(8, 128).T
    bias_pack[:, 8:12] = b2.reshape(4, 128).T
    bias_pack[:, 12:14] = b3.reshape(2, 128).T
    bias_pack[:D4, 14] = b4

    shared = {"w1": w1e, "w2": w2, "w3": w3, "w4": w4,
              "bias": bias_pack}
    in_maps = []
    for c in range(N_CORES):
        xs = np.ascontiguousarray(x[c * BC:(c + 1) * BC].T).astype(NP_BF16)  # [784, 2048]
        in_maps.append({"x": xs, **shared})
    return in_maps


def _run(inputs, trace=False):
    nc = _build_nc()
    in_maps = _prep_inputs(inputs)
    res = run_bass_kernel_spmd(nc, in_maps, core_ids=list(range(N_CORES)),
                               trace=trace)
    parts = [np.asarray(r["out"], np.float32).T for r in res.results]  # [2048, 10] each
    out = np.concatenate(parts, axis=0)                                # [16384, 10]
    return out, res


def kernel(**inputs):
    out, _ = _run(inputs, trace=False)
    return out

